# revision 10
# baseline (speedup 1.0000x reference)
"""Trainium2 Bass kernel v5 for nn_BrickVectorEdgeModel (GNN edge MLP).

Edge phase on device; node MLP, centering constants AND the d1 stage on host.
d1 (no matmul content) is precomputed per-core as [128, 4, EDGES] fp8 and
streamed chunk-by-chunk on the otherwise-idle DMA engines, freeing the DVE.

  d1  = max(u~_j + svc_i, negc_i)          host fp8, DMA-streamed
  p2  = wcb8 @ d1                          4x2 fp8 DoubleRow MMs
  e2h = max(p2_hi + lvbh_i, 0)             DVE, fp16 (hi-variance g dims)
  d2l = max(p2_lo + s1_i, s2)              DVE, fp8 (lo-variance g dims,
                                           global-centered; sa2 pre-folded
                                           into wcb8's lo columns)
  p3  = wc16 @ e2h + DR(wcc8, d2l)         per m: 2 fp16 MMs + 1 fp8 DR MM
  e3  = relu(KK*p3 + lvc)                  ACT, fp16, unsegmented
  po  = 4x col-packed (tile_position) M=2 MMs: po[32k:32k+2] = wout_k @ e3_k
  ob  = copy(po[0:98]) fp16 -> 1 wide DMA -> host picks 8 rows, sums 4
        k-partials, adds b_out

The g-dimension (rows of W_cb / contraction of W_cc) is permuted on host so
the 256 lowest-residual-variance dims take the fp8 path (~12% of residual
energy); the high half stays fp16 and needs no centering.
"""

import numpy as np
import ml_dtypes

import concourse.bass as bass
import concourse.mybir as mybir
import concourse.tile as tile
from concourse import bacc
from concourse.bass_utils import run_bass_kernel_spmd

P = 128
H = 512
B = 4
N = 192
NCORES = 8
RLOC = 96
EDGES = RLOC * N
CHUNK = 512
NCHUNK = EDGES // CHUNK      # 36
SA1 = 64.0
G8 = 2048.0                  # fp8 scale for the e3 stationary weights

F8 = ml_dtypes.float8_e4m3   # TRN float8e4 (max 240)


def _gt(v, k):
    """[k*128, n] -> [128, k, n] (partition-tiled contraction layout)."""
    return np.ascontiguousarray(v.reshape(k, P, -1).transpose(1, 0, 2))


def _to_tiles(w):
    """[K, M] stationary -> [128, K//128, M]."""
    K, M = w.shape
    return np.ascontiguousarray(w.reshape(K // P, P, M).transpose(1, 0, 2))


def _build():
    f32 = mybir.dt.float32
    f16 = mybir.dt.float16
    fp8 = mybir.dt.float8e4
    Relu = mybir.ActivationFunctionType.Relu
    Copy = mybir.ActivationFunctionType.Copy
    add = mybir.AluOpType.add
    amax = mybir.AluOpType.max
    DR = mybir.MatmulPerfMode.DoubleRow

    nc = bacc.Bacc(None, target_bir_lowering=False)
    d1p = nc.declare_dram_parameter("d1", [P, 4, EDGES], fp8, isOutput=False)
    wcb8p = nc.declare_dram_parameter("wcb8", [P, 4, H], fp8, isOutput=False)
    lvbp = nc.declare_dram_parameter("lvbh", [P, 2, RLOC], f32, isOutput=False)
    s1p = nc.declare_dram_parameter("s1l", [P, 2, RLOC], f32, isOutput=False)
    s2p = nc.declare_dram_parameter("s2l", [P, 2], f32, isOutput=False)
    lvcp = nc.declare_dram_parameter("lvc", [P, 4], f32, isOutput=False)
    wc16p = nc.declare_dram_parameter("wc16", [P, 2, H], f16, isOutput=False)
    wcc8p = nc.declare_dram_parameter("wcc8", [P, 2, H], fp8, isOutput=False)
    woutp = nc.declare_dram_parameter("wout16", [P, 4, 2], f16, isOutput=False)
    y = nc.declare_dram_parameter("y", [98, EDGES], f16, isOutput=True)

    KK = 1.0 / (G8 * SA1)

    with tile.TileContext(nc) as tc:
        with tc.tile_pool(name="wf", bufs=1) as wf, \
             tc.tile_pool(name="wr", bufs=1) as wr, \
             tc.tile_pool(name="ep", bufs=2) as ep, \
             tc.tile_pool(name="outp", bufs=4) as outp, \
             tc.tile_pool(name="psA", bufs=5, space="PSUM") as psA, \
             tc.tile_pool(name="psB", bufs=3, space="PSUM") as psB:

            # --- input DMAs in need-order on the sync queue ---
            wcb8_t = wf.tile([P, 4, H], fp8, tag="wcb8")
            lvb_t = wf.tile([P, 2, RLOC], f32, tag="lvbh")
            s1_t = wf.tile([P, 2, RLOC], f32, tag="s1l")
            s2_t = wf.tile([P, 2], f32, tag="s2l")
            lvc_t = wf.tile([P, 4], f32, tag="lvc")
            wc16_t = wf.tile([P, 2, H], f16, tag="wc16")
            wcc8_t = wf.tile([P, 2, H], fp8, tag="wcc8")
            wout_t = wf.tile([P, 4, 2], f16, tag="wout16")
            # big weights go on the (head-idle) scalar HWDGE queue, d1 stream
            # stays on sync: the two queues halve the serial DMA head
            nc.scalar.dma_start(wcb8_t[:], wcb8p[:])
            nc.scalar.dma_start(wc16_t[:], wc16p[:])
            nc.scalar.dma_start(wcc8_t[:], wcc8p[:])

            # --- PE warmup during the DMA head (HAM clock-gate to 8/8) ---
            warm = wr.tile([P, H], f16, tag="warm")
            nc.vector.memset(warm[:], 0)
            wpt = psA.tile([P, CHUNK], f32, tag="psA")
            for _ in range(12):
                nc.tensor.matmul(wpt[:], warm[:, :P], warm[:], start=True,
                                 stop=True)

            # ---- edge phase (software-pipelined: e3 lags e2 by one unit,
            # wout lags by two, so PE never waits on the current unit's DVE) ----
            units = [(cc * CHUNK, CHUNK) for cc in range(NCHUNK - 1)]
            _t0 = (NCHUNK - 1) * CHUNK
            units += [(_t0, CHUNK // 2), (_t0 + CHUNK // 2, CHUNK // 4),
                      (_t0 + 3 * CHUNK // 4, CHUNK // 4)]

            def segs(f0, cw):
                out = []
                for rl in range(f0 // N, (f0 + cw - 1) // N + 1):
                    cs = max(f0, rl * N)
                    ce = min(f0 + cw, (rl + 1) * N)
                    out.append((rl, cs, ce))
                return out

            def fetch_d1(f0, cw):
                d1 = ep.tile([P, 4, CHUNK], fp8, tag="d1", bufs=4, name="d1")
                nc.sync.dma_start(d1[:, :, :cw], d1p[:, :, f0:f0 + cw])
                return d1

            def emit_e2(f0, cw, d1):
                """e2 MMs + stage2 DVE; hi m (0,1) -> e2h, lo (2,3) -> d2l."""
                e2h = ep.tile([P, 2, CHUNK], f16, tag="e2h", bufs=4, name="e2h")
                d2l = ep.tile([P, 2, CHUNK], fp8, tag="d2l", bufs=4, name="d2l")
                for m in range(4):
                    pt = psA.tile([P, CHUNK], f32, tag="psA", name="psA")
                    for p2 in range(2):
                        nc.tensor.matmul(
                            pt[:, :cw],
                            wcb8_t[:, 2 * p2:2 * p2 + 2, m * P:(m + 1) * P],
                            d1[:, 2 * p2:2 * p2 + 2, :cw],
                            start=(p2 == 0), stop=(p2 == 1), perf_mode=DR)
                    if m < 2:
                        for rl, cs, ce in segs(f0, cw):
                            nc.vector.tensor_scalar(
                                e2h[:, m, cs - f0:ce - f0],
                                pt[:, cs - f0:ce - f0],
                                lvb_t[:, m, rl:rl + 1], 0.0, add, amax)
                    else:
                        for rl, cs, ce in segs(f0, cw):
                            nc.vector.tensor_scalar(
                                d2l[:, m - 2, cs - f0:ce - f0],
                                pt[:, cs - f0:ce - f0],
                                s1_t[:, m - 2, rl:rl + 1],
                                s2_t[:, m - 2:m - 1], add, amax)
                return e2h, d2l

            def emit_e3(f0, cw, e2h, d2l):
                """e3 = relu(KK*psum + lvc): 2 fp16 MMs + 1 fp8 DR MM per m."""
                e3 = ep.tile([P, 4, CHUNK], f16, tag="e3", bufs=4, name="e3")
                for m in range(4):
                    pt = psB.tile([P, CHUNK], f32, tag="psB")
                    for k in range(2):
                        nc.tensor.matmul(pt[:, :cw],
                                         wc16_t[:, k, m * P:(m + 1) * P],
                                         e2h[:, k, :cw], start=(k == 0),
                                         stop=False)
                    nc.tensor.matmul(pt[:, :cw],
                                     wcc8_t[:, 0:2, m * P:(m + 1) * P],
                                     d2l[:, 0:2, :cw],
                                     start=False, stop=True, perf_mode=DR)
                    nc.scalar.activation(e3[:, m, :cw], pt[:, :cw], Relu,
                                         bias=lvc_t[:, m:m + 1], scale=KK)
                return e3

            def emit_wout(f0, cw, e3):
                po = psA.tile([P, CHUNK], f32, tag="psA", name="po")
                for k in range(4):
                    nc.tensor.matmul(po[32 * k:32 * k + 2, :cw],
                                     wout_t[:, k, :],
                                     e3[:, k, :cw],
                                     start=True, stop=True,
                                     tile_position=(0, 32 * k))
                ob = outp.tile([98, CHUNK], f16, tag="ob")
                nc.scalar.activation(ob[:, :cw], po[0:98, :cw], Copy,
                                     scale=1.0)
                nc.sync.dma_start(y[:, f0:f0 + cw], ob[:, :cw])

            d1_q = [fetch_d1(*units[0]), fetch_d1(*units[1]),
                    fetch_d1(*units[2])]
            # remaining small bias DMAs after the first d1 prefetches
            nc.sync.dma_start(lvb_t[:], lvbp[:])
            nc.sync.dma_start(s1_t[:], s1p[:])
            nc.sync.dma_start(s2_t[:], s2p[:])
            nc.sync.dma_start(lvc_t[:], lvcp[:])
            nc.sync.dma_start(wout_t[:], woutp[:])

            st2_q = []   # [(f0, cw, e2h, d2l)]
            e3_q = []    # [(f0, cw, e3)]
            for cc, (f0, cw) in enumerate(units):
                d1 = d1_q.pop(0)
                if cc + 3 < len(units):
                    d1_q.append(fetch_d1(*units[cc + 3]))
                e2h, d2l = emit_e2(f0, cw, d1)
                st2_q.append((f0, cw, e2h, d2l))
                if len(e3_q) >= 2:
                    emit_wout(*e3_q.pop(0))
                if cc >= 1:
                    pf0, pcw, pe2h, pd2l = st2_q.pop(0)
                    e3_q.append((pf0, pcw, emit_e3(pf0, pcw, pe2h, pd2l)))
            while st2_q:
                pf0, pcw, pe2h, pd2l = st2_q.pop(0)
                e3_q.append((pf0, pcw, emit_e3(pf0, pcw, pe2h, pd2l)))
            while e3_q:
                emit_wout(*e3_q.pop(0))

    nc.compile()
    return nc


_cache = {}


def _get_nc():
    if "nc" not in _cache:
        _cache["nc"] = _build()
    return _cache["nc"]


def _prep_inputs(brick_vectors, xy, W_xy, b_xy, W_a, b_a, W_b, b_b,
                 W_ca, b_ca, W_cb, b_cb, W_cc, b_cc, W_out, b_out):
    brick_vectors = np.asarray(brick_vectors, np.float32)
    xy = np.asarray(xy, np.float32)
    args = [np.asarray(a, np.float32) for a in
            (W_xy, b_xy, W_a, b_a, W_b, b_b, W_ca, b_ca, W_cb, b_cb,
             W_cc, b_cc, W_out, b_out)]
    (W_xy, b_xy, W_a, b_a, W_b, b_b, W_ca, b_ca, W_cb, b_cb,
     W_cc, b_cc, W_out, b_out) = args

    # node MLP on host (0.5% of the model FLOPs; also needed for centering)
    f1 = np.maximum(
        np.einsum("bnd,hd->bnh", brick_vectors, W_a)
        + np.einsum("bnt,ht->bnh", xy, W_xy) + b_a + b_xy, 0.0)
    f2 = np.maximum(np.einsum("bnh,gh->bng", np.float16(f1).astype(np.float32),
                              np.float16(W_b).astype(np.float32)) + b_b, 0.0)
    f2 = np.float16(f2).astype(np.float32)
    u_g = np.einsum("bnh,gh->bng", f2, W_ca[:, :H])
    vpb_g = np.einsum("bnh,gh->bng", f2, W_ca[:, H:]) + b_ca

    xmax = max(float(np.abs(u_g).max() + np.abs(vpb_g).max()), 1e-3)
    sa1 = min(SA1, 2.0 ** np.floor(np.log2(200.0 / xmax)))
    assert sa1 == SA1, "sa1 changed; KK constant needs rebuild"

    # centers: c[b, i, h] = sa1 * E_j relu(u_j + v_i + b)
    c_g = np.empty((B, N, H), np.float32)
    for b in range(B):
        e1 = np.maximum(u_g[b][None, :, :] + vpb_g[b][:, None, :], 0.0)
        c_g[b] = e1.mean(axis=1)
    c_g *= sa1
    c16 = np.float16(c_g).astype(np.float32)

    # ---- sampled E2 (device arithmetic) for the g-sort, c2 and sa2 ----
    wq_cb = W_cb.astype(F8).astype(np.float32)
    rng = np.random.default_rng(12345)
    res_var = np.zeros(H, np.float64)
    E2s_all = []
    for b in range(B):
        js = rng.choice(N, 16, replace=False)
        is_ = rng.choice(N, 24, replace=False)
        e1s = np.maximum(u_g[b][js][None] + vpb_g[b][is_][:, None], 0.0)
        d1s = (sa1 * e1s - c16[b][is_][:, None]).astype(F8).astype(np.float32)
        lvbs = (W_cb @ c16[b][is_].T).T[:, None, :] + sa1 * b_cb
        E2s = np.maximum(d1s @ wq_cb.T + lvbs, 0.0)
        res_var += ((E2s - E2s.mean(axis=1, keepdims=True)) ** 2
                    ).mean(axis=(0, 1))
        E2s_all.append(E2s.reshape(-1, H))
    E2s_all = np.concatenate(E2s_all, axis=0)
    perm = np.argsort(res_var)[::-1].copy()   # hi-variance first -> fp16 half
    c2g = np.float16(E2s_all.mean(axis=0)).astype(np.float32)  # [H]

    Wb_p = W_cb[perm]
    bcb_p = b_cb[perm]
    Wc_p = W_cc[:, perm]
    c2_p = c2g[perm]
    lo = slice(256, 512)

    # sa2: fp8 range for the lo residual (2.5x margin) and the c2 shift
    R_lo = float(np.abs(E2s_all[:, perm[lo]] - c2_p[lo]).max()) * 2.5
    R_lo = max(R_lo, float(c2_p[lo].max()) * 1.2, 1e-3)
    sa2 = min(2.0 ** np.floor(np.log2(220.0 / R_lo)), 256.0)

    # stationary weights (shared across cores); sa2 folded into wcb8's lo
    # columns (power-of-2, exact in fp8) and divided out of wcc8.
    Wb_dev = Wb_p.copy()
    Wb_dev[256:] *= sa2
    wcb8 = _to_tiles(Wb_dev.T).astype(F8)                     # [128,4,512]
    wc16 = _to_tiles(G8 * Wc_p[:, :256].T).astype(np.float16)  # [128,2,512]
    wcc8 = _to_tiles((G8 / sa2) * Wc_p[:, lo].T).astype(F8)   # [128,2,512]
    wo = np.zeros((H, 2), np.float32)
    wo[:, 0:2] = W_out.T
    wout16 = _to_tiles(wo).astype(np.float16)                 # [128,4,2]

    # e3 bias (exact weights on the center): lvc = Wc_lo@c2_lo/sa1 + b_cc
    lvc_true = (Wc_p[:, lo] @ c2_p[lo]) / sa1 + b_cc          # [512]
    lvc = _gt(lvc_true.reshape(H, 1), 4).reshape(P, 4)

    perms = []
    in_maps = []
    for core in range(NCORES):
        b, half = core // 2, core % 2
        rperm = np.concatenate([np.arange(96) + 96 * half,
                                np.arange(96) + 96 * (1 - half)])
        perms.append((b, rperm))
        c16b = c16[b][rperm[:RLOC]].T                       # [512, 96]
        svc = sa1 * vpb_g[b][rperm[:RLOC]].T - c16b         # [512, 96]
        lvb_full = Wb_p @ c16b + sa1 * bcb_p[:, None]       # [512, 96]
        u16 = np.float16(sa1 * u_g[b][rperm].T
                         ).astype(np.float32)               # [512, 192]
        s1l = sa2 * (lvb_full[256:] - c2_p[lo][:, None])    # [256, 96]
        s2l = -sa2 * c2_p[lo]                               # [256]
        # d1 for all edges of this core: [512, 96, 192] -> fp8
        d1_full = np.maximum(u16[:, None, :] + svc[:, :, None],
                             -c16b[:, :, None]).astype(F8)
        in_maps.append({
            "d1": _gt(d1_full.reshape(H, EDGES), 4),
            "wcb8": wcb8,
            "lvbh": _gt(lvb_full[:256].astype(np.float32), 2),
            "s1l": _gt(s1l.astype(np.float32), 2),
            "s2l": _gt(s2l.reshape(256, 1).astype(np.float32), 2
                       ).reshape(P, 2),
            "lvc": lvc.astype(np.float32),
            "wc16": wc16,
            "wcc8": wcc8,
            "wout16": wout16,
        })
    return in_maps, perms


def kernel(brick_vectors, xy, W_xy, b_xy, W_a, b_a, W_b, b_b,
           W_ca, b_ca, W_cb, b_cb, W_cc, b_cc, W_out, b_out):
    in_maps, perms = _prep_inputs(
        brick_vectors, xy, W_xy, b_xy, W_a, b_a, W_b, b_b,
        W_ca, b_ca, W_cb, b_cb, W_cc, b_cc, W_out, b_out)
    b_out = np.asarray(b_out, np.float32)

    nc = _get_nc()
    res = run_bass_kernel_spmd(nc, in_maps, list(range(NCORES)))

    rows = [0, 1, 32, 33, 64, 65, 96, 97]
    out = np.empty((B, N, N, 2), np.float32)
    for c in range(NCORES):
        b, rperm = perms[c]
        y8 = res.results[c]["y"][rows].astype(np.float32)   # [8, EDGES]
        yc = y8.reshape(4, 2, EDGES).sum(axis=0) + b_out[:, None]
        yc = yc.reshape(2, RLOC, N)
        out[b][np.ix_(rperm[:RLOC], rperm)] = yc.transpose(1, 2, 0)
    return out


# revision 12
# speedup vs baseline: 1.0051x; 1.0051x over previous
"""Trainium2 Bass kernel v5 for nn_BrickVectorEdgeModel (GNN edge MLP).

Edge phase on device; node MLP, centering constants AND the d1 stage on host.
d1 (no matmul content) is precomputed per-core as [128, 4, EDGES] fp8 and
streamed chunk-by-chunk on the otherwise-idle DMA engines, freeing the DVE.

  d1  = max(u~_j + svc_i, negc_i)          host fp8, DMA-streamed
  p2  = wcb8 @ d1                          4x2 fp8 DoubleRow MMs
  e2h = max(p2_hi + lvbh_i, 0)             DVE, fp16 (hi-variance g dims)
  d2l = max(p2_lo + s1_i, s2)              DVE, fp8 (lo-variance g dims,
                                           global-centered; sa2 pre-folded
                                           into wcb8's lo columns)
  p3  = wc16 @ e2h + DR(wcc8, d2l)         per m: 2 fp16 MMs + 1 fp8 DR MM
  e3  = relu(KK*p3 + lvc)                  ACT, fp16, unsegmented
  po  = 4x col-packed (tile_position) M=2 MMs: po[32k:32k+2] = wout_k @ e3_k
  ob  = copy(po[0:98]) fp16 -> 1 wide DMA -> host picks 8 rows, sums 4
        k-partials, adds b_out

The g-dimension (rows of W_cb / contraction of W_cc) is permuted on host so
the 256 lowest-residual-variance dims take the fp8 path (~12% of residual
energy); the high half stays fp16 and needs no centering.
"""

import numpy as np
import ml_dtypes

import concourse.bass as bass
import concourse.mybir as mybir
import concourse.tile as tile
from concourse import bacc
from concourse.bass_utils import run_bass_kernel_spmd

P = 128
H = 512
B = 4
N = 192
NCORES = 8
RLOC = 96
EDGES = RLOC * N
CHUNK = 512
NCHUNK = EDGES // CHUNK      # 36
SA1 = 64.0
G8 = 2048.0                  # fp8 scale for the e3 stationary weights

F8 = ml_dtypes.float8_e4m3   # TRN float8e4 (max 240)


def _gt(v, k):
    """[k*128, n] -> [128, k, n] (partition-tiled contraction layout)."""
    return np.ascontiguousarray(v.reshape(k, P, -1).transpose(1, 0, 2))


def _to_tiles(w):
    """[K, M] stationary -> [128, K//128, M]."""
    K, M = w.shape
    return np.ascontiguousarray(w.reshape(K // P, P, M).transpose(1, 0, 2))


def _build():
    f32 = mybir.dt.float32
    f16 = mybir.dt.float16
    fp8 = mybir.dt.float8e4
    Relu = mybir.ActivationFunctionType.Relu
    Copy = mybir.ActivationFunctionType.Copy
    add = mybir.AluOpType.add
    amax = mybir.AluOpType.max
    DR = mybir.MatmulPerfMode.DoubleRow

    nc = bacc.Bacc(None, target_bir_lowering=False)
    d1p = nc.declare_dram_parameter("d1", [P, 4, EDGES], fp8, isOutput=False)
    wcb8p = nc.declare_dram_parameter("wcb8", [P, 4, H], fp8, isOutput=False)
    lvbp = nc.declare_dram_parameter("lvbh", [P, 2, RLOC], f32, isOutput=False)
    s1p = nc.declare_dram_parameter("s1l", [P, 2, RLOC], f32, isOutput=False)
    s2p = nc.declare_dram_parameter("s2l", [P, 2], f32, isOutput=False)
    lvcp = nc.declare_dram_parameter("lvc", [P, 4], f32, isOutput=False)
    wc16p = nc.declare_dram_parameter("wc16", [P, 2, H], f16, isOutput=False)
    wcc8p = nc.declare_dram_parameter("wcc8", [P, 2, H], fp8, isOutput=False)
    woutp = nc.declare_dram_parameter("wout16", [P, 4, 2], f16, isOutput=False)
    y = nc.declare_dram_parameter("y", [98, EDGES], f16, isOutput=True)

    KK = 1.0 / (G8 * SA1)

    with tile.TileContext(nc) as tc:
        with tc.tile_pool(name="wf", bufs=1) as wf, \
             tc.tile_pool(name="wr", bufs=1) as wr, \
             tc.tile_pool(name="ep", bufs=2) as ep, \
             tc.tile_pool(name="outp", bufs=4) as outp, \
             tc.tile_pool(name="psA", bufs=5, space="PSUM") as psA, \
             tc.tile_pool(name="psB", bufs=3, space="PSUM") as psB:

            # --- input DMAs in need-order on the sync queue ---
            wcb8_t = wf.tile([P, 4, H], fp8, tag="wcb8")
            lvb_t = wf.tile([P, 2, RLOC], f32, tag="lvbh")
            s1_t = wf.tile([P, 2, RLOC], f32, tag="s1l")
            s2_t = wf.tile([P, 2], f32, tag="s2l")
            lvc_t = wf.tile([P, 4], f32, tag="lvc")
            wc16_t = wf.tile([P, 2, H], f16, tag="wc16")
            wcc8_t = wf.tile([P, 2, H], fp8, tag="wcc8")
            wout_t = wf.tile([P, 4, 2], f16, tag="wout16")
            # big weights go on the (head-idle) scalar HWDGE queue, d1 stream
            # stays on sync: the two queues halve the serial DMA head
            nc.scalar.dma_start(wcb8_t[:], wcb8p[:])
            nc.scalar.dma_start(wc16_t[:], wc16p[:])
            nc.scalar.dma_start(wcc8_t[:], wcc8p[:])

            # --- small PE warmup; input DMAs complete during the framework
            # preamble, so the first real MMs take over HAM warming ---
            warm = wr.tile([P, H], f16, tag="warm")
            nc.gpsimd.memset(warm[:], 0)
            wpt = psA.tile([P, CHUNK], f32, tag="psA")
            for _ in range(4):
                nc.tensor.matmul(wpt[:], warm[:, :P], warm[:], start=True,
                                 stop=True)

            # ---- edge phase (software-pipelined: e3 lags e2 by one unit,
            # wout lags by two, so PE never waits on the current unit's DVE) ----
            units = [(cc * CHUNK, CHUNK) for cc in range(NCHUNK - 1)]
            _t0 = (NCHUNK - 1) * CHUNK
            units += [(_t0, CHUNK // 2), (_t0 + CHUNK // 2, CHUNK // 2)]

            def segs(f0, cw):
                out = []
                for rl in range(f0 // N, (f0 + cw - 1) // N + 1):
                    cs = max(f0, rl * N)
                    ce = min(f0 + cw, (rl + 1) * N)
                    out.append((rl, cs, ce))
                return out

            def fetch_d1(f0, cw):
                d1 = ep.tile([P, 4, CHUNK], fp8, tag="d1", bufs=4, name="d1")
                nc.sync.dma_start(d1[:, :, :cw], d1p[:, :, f0:f0 + cw])
                return d1

            def emit_e2(f0, cw, d1):
                """e2 MMs + stage2 DVE; hi m (0,1) -> e2h, lo (2,3) -> d2l."""
                e2h = ep.tile([P, 2, CHUNK], f16, tag="e2h", bufs=4, name="e2h")
                d2l = ep.tile([P, 2, CHUNK], fp8, tag="d2l", bufs=4, name="d2l")
                for m in range(4):
                    pt = psA.tile([P, CHUNK], f32, tag="psA", name="psA")
                    for p2 in range(2):
                        nc.tensor.matmul(
                            pt[:, :cw],
                            wcb8_t[:, 2 * p2:2 * p2 + 2, m * P:(m + 1) * P],
                            d1[:, 2 * p2:2 * p2 + 2, :cw],
                            start=(p2 == 0), stop=(p2 == 1), perf_mode=DR)
                    if m < 2:
                        for rl, cs, ce in segs(f0, cw):
                            nc.vector.tensor_scalar(
                                e2h[:, m, cs - f0:ce - f0],
                                pt[:, cs - f0:ce - f0],
                                lvb_t[:, m, rl:rl + 1], 0.0, add, amax)
                    else:
                        for rl, cs, ce in segs(f0, cw):
                            nc.vector.tensor_scalar(
                                d2l[:, m - 2, cs - f0:ce - f0],
                                pt[:, cs - f0:ce - f0],
                                s1_t[:, m - 2, rl:rl + 1],
                                s2_t[:, m - 2:m - 1], add, amax)
                return e2h, d2l

            def emit_e3(f0, cw, e2h, d2l):
                """e3 = relu(KK*psum + lvc): 2 fp16 MMs + 1 fp8 DR MM per m."""
                e3 = ep.tile([P, 4, CHUNK], f16, tag="e3", bufs=4, name="e3")
                for m in range(4):
                    pt = psB.tile([P, CHUNK], f32, tag="psB")
                    for k in range(2):
                        nc.tensor.matmul(pt[:, :cw],
                                         wc16_t[:, k, m * P:(m + 1) * P],
                                         e2h[:, k, :cw], start=(k == 0),
                                         stop=False)
                    nc.tensor.matmul(pt[:, :cw],
                                     wcc8_t[:, 0:2, m * P:(m + 1) * P],
                                     d2l[:, 0:2, :cw],
                                     start=False, stop=True, perf_mode=DR)
                    nc.scalar.activation(e3[:, m, :cw], pt[:, :cw], Relu,
                                         bias=lvc_t[:, m:m + 1], scale=KK)
                return e3

            def emit_wout(f0, cw, e3):
                po = psA.tile([P, CHUNK], f32, tag="psA", name="po")
                for k in range(4):
                    nc.tensor.matmul(po[32 * k:32 * k + 2, :cw],
                                     wout_t[:, k, :],
                                     e3[:, k, :cw],
                                     start=True, stop=True,
                                     tile_position=(0, 32 * k))
                ob = outp.tile([98, CHUNK], f16, tag="ob")
                nc.scalar.activation(ob[:, :cw], po[0:98, :cw], Copy,
                                     scale=1.0)
                nc.sync.dma_start(y[:, f0:f0 + cw], ob[:, :cw])

            d1_q = [fetch_d1(*units[0]), fetch_d1(*units[1]),
                    fetch_d1(*units[2])]
            # remaining small bias DMAs after the first d1 prefetches
            nc.sync.dma_start(lvb_t[:], lvbp[:])
            nc.sync.dma_start(s1_t[:], s1p[:])
            nc.sync.dma_start(s2_t[:], s2p[:])
            nc.sync.dma_start(lvc_t[:], lvcp[:])
            nc.sync.dma_start(wout_t[:], woutp[:])

            st2_q = []   # [(f0, cw, e2h, d2l)]
            e3_q = []    # [(f0, cw, e3)]
            for cc, (f0, cw) in enumerate(units):
                d1 = d1_q.pop(0)
                if cc + 3 < len(units):
                    d1_q.append(fetch_d1(*units[cc + 3]))
                e2h, d2l = emit_e2(f0, cw, d1)
                st2_q.append((f0, cw, e2h, d2l))
                if len(e3_q) >= 2:
                    emit_wout(*e3_q.pop(0))
                if cc >= 1:
                    pf0, pcw, pe2h, pd2l = st2_q.pop(0)
                    e3_q.append((pf0, pcw, emit_e3(pf0, pcw, pe2h, pd2l)))
            while st2_q:
                pf0, pcw, pe2h, pd2l = st2_q.pop(0)
                e3_q.append((pf0, pcw, emit_e3(pf0, pcw, pe2h, pd2l)))
            while e3_q:
                emit_wout(*e3_q.pop(0))

    nc.compile()
    return nc


_cache = {}


def _get_nc():
    if "nc" not in _cache:
        _cache["nc"] = _build()
    return _cache["nc"]


def _prep_inputs(brick_vectors, xy, W_xy, b_xy, W_a, b_a, W_b, b_b,
                 W_ca, b_ca, W_cb, b_cb, W_cc, b_cc, W_out, b_out):
    brick_vectors = np.asarray(brick_vectors, np.float32)
    xy = np.asarray(xy, np.float32)
    args = [np.asarray(a, np.float32) for a in
            (W_xy, b_xy, W_a, b_a, W_b, b_b, W_ca, b_ca, W_cb, b_cb,
             W_cc, b_cc, W_out, b_out)]
    (W_xy, b_xy, W_a, b_a, W_b, b_b, W_ca, b_ca, W_cb, b_cb,
     W_cc, b_cc, W_out, b_out) = args

    # node MLP on host (0.5% of the model FLOPs; also needed for centering)
    f1 = np.maximum(
        np.einsum("bnd,hd->bnh", brick_vectors, W_a)
        + np.einsum("bnt,ht->bnh", xy, W_xy) + b_a + b_xy, 0.0)
    f2 = np.maximum(np.einsum("bnh,gh->bng", np.float16(f1).astype(np.float32),
                              np.float16(W_b).astype(np.float32)) + b_b, 0.0)
    f2 = np.float16(f2).astype(np.float32)
    u_g = np.einsum("bnh,gh->bng", f2, W_ca[:, :H])
    vpb_g = np.einsum("bnh,gh->bng", f2, W_ca[:, H:]) + b_ca

    xmax = max(float(np.abs(u_g).max() + np.abs(vpb_g).max()), 1e-3)
    sa1 = min(SA1, 2.0 ** np.floor(np.log2(200.0 / xmax)))
    assert sa1 == SA1, "sa1 changed; KK constant needs rebuild"

    # centers: c[b, i, h] = sa1 * E_j relu(u_j + v_i + b)
    c_g = np.empty((B, N, H), np.float32)
    for b in range(B):
        e1 = np.maximum(u_g[b][None, :, :] + vpb_g[b][:, None, :], 0.0)
        c_g[b] = e1.mean(axis=1)
    c_g *= sa1
    c16 = np.float16(c_g).astype(np.float32)

    # ---- sampled E2 (device arithmetic) for the g-sort, c2 and sa2 ----
    wq_cb = W_cb.astype(F8).astype(np.float32)
    rng = np.random.default_rng(12345)
    res_var = np.zeros(H, np.float64)
    E2s_all = []
    for b in range(B):
        js = rng.choice(N, 16, replace=False)
        is_ = rng.choice(N, 24, replace=False)
        e1s = np.maximum(u_g[b][js][None] + vpb_g[b][is_][:, None], 0.0)
        d1s = (sa1 * e1s - c16[b][is_][:, None]).astype(F8).astype(np.float32)
        lvbs = (W_cb @ c16[b][is_].T).T[:, None, :] + sa1 * b_cb
        E2s = np.maximum(d1s @ wq_cb.T + lvbs, 0.0)
        res_var += ((E2s - E2s.mean(axis=1, keepdims=True)) ** 2
                    ).mean(axis=(0, 1))
        E2s_all.append(E2s.reshape(-1, H))
    E2s_all = np.concatenate(E2s_all, axis=0)
    perm = np.argsort(res_var)[::-1].copy()   # hi-variance first -> fp16 half
    c2g = np.float16(E2s_all.mean(axis=0)).astype(np.float32)  # [H]

    Wb_p = W_cb[perm]
    bcb_p = b_cb[perm]
    Wc_p = W_cc[:, perm]
    c2_p = c2g[perm]
    lo = slice(256, 512)

    # sa2: fp8 range for the lo residual (2.5x margin) and the c2 shift
    R_lo = float(np.abs(E2s_all[:, perm[lo]] - c2_p[lo]).max()) * 2.5
    R_lo = max(R_lo, float(c2_p[lo].max()) * 1.2, 1e-3)
    sa2 = min(2.0 ** np.floor(np.log2(220.0 / R_lo)), 256.0)

    # stationary weights (shared across cores); sa2 folded into wcb8's lo
    # columns (power-of-2, exact in fp8) and divided out of wcc8.
    Wb_dev = Wb_p.copy()
    Wb_dev[256:] *= sa2
    wcb8 = _to_tiles(Wb_dev.T).astype(F8)                     # [128,4,512]
    wc16 = _to_tiles(G8 * Wc_p[:, :256].T).astype(np.float16)  # [128,2,512]
    wcc8 = _to_tiles((G8 / sa2) * Wc_p[:, lo].T).astype(F8)   # [128,2,512]
    wo = np.zeros((H, 2), np.float32)
    wo[:, 0:2] = W_out.T
    wout16 = _to_tiles(wo).astype(np.float16)                 # [128,4,2]

    # e3 bias (exact weights on the center): lvc = Wc_lo@c2_lo/sa1 + b_cc
    lvc_true = (Wc_p[:, lo] @ c2_p[lo]) / sa1 + b_cc          # [512]
    lvc = _gt(lvc_true.reshape(H, 1), 4).reshape(P, 4)

    perms = []
    in_maps = []
    for core in range(NCORES):
        b, half = core // 2, core % 2
        rperm = np.concatenate([np.arange(96) + 96 * half,
                                np.arange(96) + 96 * (1 - half)])
        perms.append((b, rperm))
        c16b = c16[b][rperm[:RLOC]].T                       # [512, 96]
        svc = sa1 * vpb_g[b][rperm[:RLOC]].T - c16b         # [512, 96]
        lvb_full = Wb_p @ c16b + sa1 * bcb_p[:, None]       # [512, 96]
        u16 = np.float16(sa1 * u_g[b][rperm].T
                         ).astype(np.float32)               # [512, 192]
        s1l = sa2 * (lvb_full[256:] - c2_p[lo][:, None])    # [256, 96]
        s2l = -sa2 * c2_p[lo]                               # [256]
        # d1 for all edges of this core: [512, 96, 192] -> fp8
        d1_full = np.maximum(u16[:, None, :] + svc[:, :, None],
                             -c16b[:, :, None]).astype(F8)
        in_maps.append({
            "d1": _gt(d1_full.reshape(H, EDGES), 4),
            "wcb8": wcb8,
            "lvbh": _gt(lvb_full[:256].astype(np.float32), 2),
            "s1l": _gt(s1l.astype(np.float32), 2),
            "s2l": _gt(s2l.reshape(256, 1).astype(np.float32), 2
                       ).reshape(P, 2),
            "lvc": lvc.astype(np.float32),
            "wc16": wc16,
            "wcc8": wcc8,
            "wout16": wout16,
        })
    return in_maps, perms


def kernel(brick_vectors, xy, W_xy, b_xy, W_a, b_a, W_b, b_b,
           W_ca, b_ca, W_cb, b_cb, W_cc, b_cc, W_out, b_out):
    in_maps, perms = _prep_inputs(
        brick_vectors, xy, W_xy, b_xy, W_a, b_a, W_b, b_b,
        W_ca, b_ca, W_cb, b_cb, W_cc, b_cc, W_out, b_out)
    b_out = np.asarray(b_out, np.float32)

    nc = _get_nc()
    res = run_bass_kernel_spmd(nc, in_maps, list(range(NCORES)))

    rows = [0, 1, 32, 33, 64, 65, 96, 97]
    out = np.empty((B, N, N, 2), np.float32)
    for c in range(NCORES):
        b, rperm = perms[c]
        y8 = res.results[c]["y"][rows].astype(np.float32)   # [8, EDGES]
        yc = y8.reshape(4, 2, EDGES).sum(axis=0) + b_out[:, None]
        yc = yc.reshape(2, RLOC, N)
        out[b][np.ix_(rperm[:RLOC], rperm)] = yc.transpose(1, 2, 0)
    return out
